# revision 5
# baseline (speedup 1.0000x reference)
"""DRAE loss kernel for 8 Trainium2 NeuronCores.

Computes: per-sample squared error Err[n] = sum((input-target)^2, dim=1),
then an Otsu-style optimal inlier/outlier split over sorted Err, returning
mean(inlier Err) + LAMB * min_obj.

Strategy (single SPMD launch over 8 cores):
  - rows sharded 1024/core; Err computed per-core (DMA-bound phase)
  - centered v = Err - 8192 (exact in fp32 for this data range)
  - AllGather the 8 x 1024 centered-Err chunks -> full v[8192] on every core
  - threshold search without sorting: for each local candidate t_k,
    (n1, c1) = (count, sum) of {v_j <= t_k} via a 0/1 comparison matrix
    (VectorE) contracted on TensorE with stationary [1, v_hi, v_lo] (bf16
    hi/lo split of v, exact to ~17 mantissa bits)
  - Sw1+Sw2 = totq - c1^2/n1 - (tot-c1)^2/(N-n1): maximize
    U = c1^2/n1 + (tot-c1)^2/(N-n1) with smallest-n1 tiebreak
  - AllGather per-core (Umax, n1, c1) triples, final scalar replicated
"""

import numpy as np

N = 8192
D = 4096
NCORES = 8
LOCAL = N // NCORES        # 1024 rows per core
RTILES = LOCAL // 128      # 8 row tiles per core
JT = N // 128              # 64 comparison tiles
SHIFT = 8192.0
LAMB = 0.1
BIG = 1.0e30

_CACHE = {}


def _build():
    import concourse.bass as bass
    import concourse.tile as tile
    from concourse import bacc, mybir
    from concourse import bass_isa

    f32 = mybir.dt.float32
    bf16 = mybir.dt.bfloat16
    Alu = mybir.AluOpType
    Act = mybir.ActivationFunctionType

    nc = bacc.Bacc("TRN2", target_bir_lowering=False, debug=False,
                   num_devices=NCORES)

    x_in = nc.dram_tensor("x", [LOCAL, D], f32, kind="ExternalInput")
    t_in = nc.dram_tensor("t", [LOCAL, D], f32, kind="ExternalInput")
    out_d = nc.dram_tensor("out", [1, 1], f32, kind="ExternalOutput")

    with tile.TileContext(nc) as tc:
        with tc.tile_pool(name="io", bufs=3) as io_pool, \
             tc.tile_pool(name="sm", bufs=1) as sm, \
             tc.tile_pool(name="cmp", bufs=4) as cmp_pool, \
             tc.tile_pool(name="ps", bufs=1, space="PSUM") as ps, \
             tc.tile_pool(name="dram", bufs=1, space="DRAM") as dram:

            # ---------------- phase 1: per-core Err ----------------
            err_sb = sm.tile([128, RTILES], f32)        # accum cols per tile
            xv = x_in.ap().rearrange("(r p) d -> r p d", p=128)
            tv = t_in.ap().rearrange("(r p) d -> r p d", p=128)
            for r in range(RTILES):
                xt = io_pool.tile([128, D], f32, tag="xt")
                tt = io_pool.tile([128, D], f32, tag="tt")
                nc.sync.dma_start(xt[:], xv[r])
                nc.sync.dma_start(tt[:], tv[r])
                # d = x - t (in place over xt)
                nc.vector.tensor_tensor(xt[:], xt[:], tt[:], Alu.subtract)
                # rowsum(d^2) on ScalarE; full-size out is junk (reuses tt)
                nc.scalar.activation(tt[:], xt[:], Act.Square,
                                     accum_out=err_sb[:, r:r + 1])
            # center: v = Err - 8192 (exact)
            v_sb = sm.tile([128, RTILES], f32)
            nc.vector.tensor_scalar(v_sb[:], err_sb[:], -SHIFT, None, Alu.add)

            # write local centered errs p-major: err_loc[p*8 + r] = v_sb[p, r]
            err_loc = dram.tile([1, LOCAL], f32)
            nc.sync.dma_start(
                err_loc[:].rearrange("one (p r) -> (one p) r", p=128), v_sb[:])

            # ---------------- allgather #1 ----------------
            vg = dram.tile([NCORES, LOCAL], f32)
            nc.gpsimd.collective_compute(
                "AllGather", Alu.bypass,
                replica_groups=[list(range(NCORES))],
                ins=[err_loc[:].opt()], outs=[vg[:].opt()],
            )

            # ---------------- phase 2 prep ----------------
            # Vsb[p, jt=c*8+r] = v_global[c*1024 + p*8 + r]
            vsb = sm.tile([128, JT], f32)
            nc.sync.dma_start(
                vsb[:].rearrange("p (c r) -> p c r", c=NCORES),
                vg[:].rearrange("c (p r) -> p c r", p=128))

            # candidates = this core's own chunk, broadcast along partitions
            crow = sm.tile([1, LOCAL], f32)
            nc.sync.dma_start(crow[:], err_loc[:])
            tb = sm.tile([128, LOCAL], f32)
            nc.gpsimd.partition_broadcast(tb[:], crow[:])

            # V3 stationary: per j-tile columns [1.0, v_hi, v_lo] (bf16)
            v3 = sm.tile([128, 3 * JT], bf16)
            v3v = v3[:].rearrange("p (j c) -> p j c", c=3)
            nc.vector.memset(v3v[:, :, 0], 1.0)
            nc.vector.tensor_copy(v3v[:, :, 1], vsb[:])          # v_hi (cast)
            vhi32 = sm.tile([128, JT], f32)
            nc.vector.tensor_copy(vhi32[:], v3v[:, :, 1])        # back to f32
            vlo32 = sm.tile([128, JT], f32)
            nc.vector.tensor_tensor(vlo32[:], vsb[:], vhi32[:], Alu.subtract)
            nc.vector.tensor_copy(v3v[:, :, 2], vlo32[:])        # v_lo (cast)

            # tot / totq scalars (partition sums via tiny fp32 matmuls)
            junk = sm.tile([128, JT], f32, tag="junk")
            trow = sm.tile([128, 1], f32)
            nc.scalar.activation(junk[:], vsb[:], Act.Copy, accum_out=trow[:])
            junk2 = sm.tile([128, JT], f32, tag="junk2")
            qrow = sm.tile([128, 1], f32)
            nc.scalar.activation(junk2[:], vsb[:], Act.Square, accum_out=qrow[:])
            ones_col = sm.tile([128, 1], f32)
            nc.vector.memset(ones_col[:], 1.0)
            tot_ps = ps.tile([1, 1], f32, tag="tot")
            totq_ps = ps.tile([1, 1], f32, tag="totq")
            nc.tensor.matmul(tot_ps[:], trow[:], ones_col[:], start=True, stop=True)
            nc.tensor.matmul(totq_ps[:], qrow[:], ones_col[:], start=True, stop=True)
            tot_sb = sm.tile([1, 2], f32)
            nc.vector.tensor_copy(tot_sb[:, 0:1], tot_ps[:])
            nc.vector.tensor_copy(tot_sb[:, 1:2], totq_ps[:])
            tot_b = sm.tile([128, 1], f32)
            nc.gpsimd.partition_broadcast(tot_b[:], tot_sb[:, 0:1])

            # ---------------- phase 2 main: compare + matmul ----------------
            p0 = ps.tile([3, 512], f32, tag="p0")
            p1 = ps.tile([3, 512], f32, tag="p1")
            for j in range(JT):
                mj = cmp_pool.tile([128, LOCAL], bf16, tag="mj")
                nc.vector.tensor_scalar(mj[:], tb[:], vsb[:, j:j + 1], None,
                                        Alu.is_ge)
                lhs = v3v[:, j, :]
                nc.tensor.matmul(p0[:], lhs, mj[:, 0:512],
                                 start=(j == 0), stop=(j == JT - 1))
                nc.tensor.matmul(p1[:], lhs, mj[:, 512:1024],
                                 start=(j == 0), stop=(j == JT - 1))

            # ---------------- phase 3: per-candidate stats ----------------
            # copy psum -> sbuf, bounce via DRAM to scatter rows across
            # partitions: [3, 1024] -> three [128, 8] tiles (k = p*8 + f)
            stage = sm.tile([3, 1024], f32)
            nc.vector.tensor_copy(stage[:, 0:512], p0[:])
            nc.vector.tensor_copy(stage[:, 512:1024], p1[:])
            stg_d = dram.tile([3, 1024], f32)
            nc.sync.dma_start(stg_d[:], stage[:])
            n1 = sm.tile([128, 8], f32)
            chi = sm.tile([128, 8], f32)
            clo = sm.tile([128, 8], f32)
            for row, dst in ((0, n1), (1, chi), (2, clo)):
                nc.sync.dma_start(
                    dst[:],
                    stg_d[row:row + 1, :].rearrange(
                        "one (p f) -> (one p) f", f=8))
            c1 = sm.tile([128, 8], f32)
            nc.vector.tensor_tensor(c1[:], chi[:], clo[:], Alu.add)

            # U = c1^2/n1 + (tot-c1)^2/(N-n1)
            rn1 = sm.tile([128, 8], f32)
            nc.vector.reciprocal(rn1[:], n1[:])
            t1 = sm.tile([128, 8], f32, tag="t1")
            nc.vector.tensor_tensor(t1[:], c1[:], c1[:], Alu.mult)
            nc.vector.tensor_tensor(t1[:], t1[:], rn1[:], Alu.mult)
            r2 = sm.tile([128, 8], f32, tag="r2")
            nc.vector.tensor_scalar(r2[:], c1[:], tot_b[:], -1.0,
                                    Alu.subtract, Alu.mult)
            n2 = sm.tile([128, 8], f32, tag="n2")
            nc.vector.tensor_scalar(n2[:], n1[:], float(N), -1.0,
                                    Alu.subtract, Alu.mult)
            rn2 = sm.tile([128, 8], f32, tag="rn2")
            nc.vector.reciprocal(rn2[:], n2[:])
            t3 = sm.tile([128, 8], f32, tag="t3")
            nc.vector.tensor_tensor(t3[:], r2[:], r2[:], Alu.mult)
            nc.vector.tensor_tensor(t3[:], t3[:], rn2[:], Alu.mult)
            u = sm.tile([128, 8], f32)
            nc.vector.tensor_tensor(u[:], t1[:], t3[:], Alu.add)
            # mask out the full split (n1 == N): U <- -BIG
            nbig = sm.tile([128, 8], f32, tag="nbig")
            nc.vector.memset(nbig[:], -BIG)
            mfull = sm.tile([128, 8], mybir.dt.uint8, tag="mfull")
            nc.vector.tensor_scalar(mfull[:], n1[:], float(N), None, Alu.is_ge)
            nc.vector.copy_predicated(u[:], mfull[:], nbig[:])

            # local argmax with smallest-n1 tiebreak
            def pmax(src_col, tag):
                # all-partition max of a [128,1] column -> [128,1] broadcast
                dst = sm.tile([128, 1], f32, tag=tag)
                nc.gpsimd.partition_all_reduce(dst[:], src_col[:], 128,
                                               bass_isa.ReduceOp.max)
                return dst

            rowm = sm.tile([128, 1], f32, tag="rowm")
            nc.vector.tensor_reduce(rowm[:], u[:], mybir.AxisListType.X, Alu.max)
            umax = pmax(rowm, "umax")

            msel = sm.tile([128, 8], mybir.dt.uint8, tag="msel")
            nc.vector.tensor_scalar(msel[:], u[:], umax[:], None, Alu.is_ge)
            big = sm.tile([128, 8], f32, tag="big")
            nc.vector.memset(big[:], BIG)
            sel = sm.tile([128, 8], f32, tag="sel")
            nc.vector.tensor_copy(sel[:], big[:])
            nc.vector.copy_predicated(sel[:], msel[:], n1[:])
            nc.vector.tensor_scalar(sel[:], sel[:], -1.0, None, Alu.mult)
            rown = sm.tile([128, 1], f32, tag="rown")
            nc.vector.tensor_reduce(rown[:], sel[:], mybir.AxisListType.X, Alu.max)
            n1m = pmax(rown, "n1m")  # = -min(n1 over selected)

            negn1 = sm.tile([128, 8], f32, tag="negn1")
            nc.vector.tensor_scalar(negn1[:], n1[:], -1.0, None, Alu.mult)
            m2 = sm.tile([128, 8], mybir.dt.uint8, tag="m2")
            # (-n1) >= max(-n1 over selected)  <=>  n1 <= min-selected-n1
            nc.vector.tensor_scalar(m2[:], negn1[:], n1m[:], None, Alu.is_ge)
            nc.vector.tensor_tensor(m2[:], m2[:], msel[:], Alu.logical_and)
            sel2 = sm.tile([128, 8], f32, tag="sel2")
            nc.vector.tensor_copy(sel2[:], big[:])
            nc.vector.copy_predicated(sel2[:], m2[:], c1[:])
            nc.vector.tensor_scalar(sel2[:], sel2[:], -1.0, None, Alu.mult)
            rowc = sm.tile([128, 1], f32, tag="rowc")
            nc.vector.tensor_reduce(rowc[:], sel2[:], mybir.AxisListType.X, Alu.max)
            c1m = pmax(rowc, "c1m")  # = -c1_at_opt

            # pack [U, -n1min*-1? ...] -> [umax, n1, c1, tot, totq, 0,0,0]
            pack = sm.tile([1, 8], f32)
            nc.vector.memset(pack[:], 0.0)
            nc.vector.tensor_copy(pack[:, 0:1], umax[0:1, :])
            nc.vector.tensor_scalar(pack[:, 1:2], n1m[0:1, :], -1.0, None, Alu.mult)
            nc.vector.tensor_scalar(pack[:, 2:3], c1m[0:1, :], -1.0, None, Alu.mult)
            nc.vector.tensor_copy(pack[:, 3:4], tot_sb[:, 0:1])
            nc.vector.tensor_copy(pack[:, 4:5], tot_sb[:, 1:2])

            tri_loc = dram.tile([1, 8], f32)
            nc.sync.dma_start(tri_loc[:], pack[:])
            tri_all = dram.tile([NCORES, 8], f32)
            nc.gpsimd.collective_compute(
                "AllGather", Alu.bypass,
                replica_groups=[list(range(NCORES))],
                ins=[tri_loc[:].opt()], outs=[tri_all[:].opt()],
            )

            # ---------------- final (replicated on every core) ----------------
            fa = sm.tile([1, NCORES, 8], f32)
            nc.sync.dma_start(
                fa[:], tri_all[:].rearrange("(one c) k -> one c k", one=1))
            uv = fa[:, :, 0]
            n1v = fa[:, :, 1]
            c1v = fa[:, :, 2]
            totv = fa[:, :, 3]
            totqv = fa[:, :, 4]

            gu = sm.tile([1, 1], f32, tag="gu")
            nc.vector.tensor_reduce(gu[:], uv, mybir.AxisListType.X, Alu.max)
            m8 = sm.tile([1, NCORES], mybir.dt.uint8, tag="m8")
            nc.vector.tensor_scalar(m8[:], uv, gu[:], None, Alu.is_ge)
            big8 = sm.tile([1, NCORES], f32, tag="big8")
            nc.vector.memset(big8[:], BIG)
            s8 = sm.tile([1, NCORES], f32, tag="s8")
            nc.vector.tensor_copy(s8[:], big8[:])
            nc.vector.copy_predicated(s8[:], m8[:], n1v)
            gn1 = sm.tile([1, 1], f32, tag="gn1")
            nc.vector.tensor_reduce(gn1[:], s8[:], mybir.AxisListType.X, Alu.min)
            m82 = sm.tile([1, NCORES], mybir.dt.uint8, tag="m82")
            nc.vector.tensor_scalar(m82[:], n1v, gn1[:], None, Alu.is_le)
            nc.vector.tensor_tensor(m82[:], m82[:], m8[:], Alu.logical_and)
            s82 = sm.tile([1, NCORES], f32, tag="s82")
            nc.vector.tensor_copy(s82[:], big8[:])
            nc.vector.copy_predicated(s82[:], m82[:], c1v)
            gc1 = sm.tile([1, 1], f32, tag="gc1")
            nc.vector.tensor_reduce(gc1[:], s82[:], mybir.AxisListType.X, Alu.min)

            # final = gc1/gn1 + SHIFT + LAMB*(totq - gU)/(totq - tot^2/N)
            sc = sm.tile([1, 8], f32, tag="sc")
            t0 = fa[:, 0, :]  # row with core0's [u,n1,c1,tot,totq,...]
            # Sb = totq - tot*tot/N
            nc.vector.tensor_tensor(sc[:, 0:1], t0[:, 3:4], t0[:, 3:4], Alu.mult)
            nc.vector.tensor_scalar(sc[:, 0:1], sc[:, 0:1], 1.0 / N, None, Alu.mult)
            nc.vector.tensor_tensor(sc[:, 1:2], t0[:, 4:5], sc[:, 0:1], Alu.subtract)
            # W = totq - gU ; obj = W / Sb
            nc.vector.tensor_tensor(sc[:, 2:3], t0[:, 4:5], gu[:], Alu.subtract)
            nc.vector.reciprocal(sc[:, 3:4], sc[:, 1:2])
            nc.vector.tensor_tensor(sc[:, 2:3], sc[:, 2:3], sc[:, 3:4], Alu.mult)
            # mean1 = gc1 / gn1
            nc.vector.reciprocal(sc[:, 4:5], gn1[:])
            nc.vector.tensor_tensor(sc[:, 5:6], gc1[:], sc[:, 4:5], Alu.mult)
            # out = mean1 + SHIFT + LAMB*obj
            nc.vector.tensor_scalar(sc[:, 2:3], sc[:, 2:3], LAMB, SHIFT,
                                    Alu.mult, Alu.add)
            nc.vector.tensor_tensor(sc[:, 6:7], sc[:, 5:6], sc[:, 2:3], Alu.add)
            nc.sync.dma_start(out_d.ap(), sc[:, 6:7])

    nc.compile()
    return nc


def _get_nc():
    if "nc" not in _CACHE:
        _CACHE["nc"] = _build()
    return _CACHE["nc"]


def kernel(input, target, _trace=False):
    from concourse.bass_utils import run_bass_kernel_spmd

    nc = _get_nc()
    x = np.ascontiguousarray(np.asarray(input, dtype=np.float32))
    t = np.ascontiguousarray(np.asarray(target, dtype=np.float32))
    in_maps = [
        {"x": x[c * LOCAL:(c + 1) * LOCAL], "t": t[c * LOCAL:(c + 1) * LOCAL]}
        for c in range(NCORES)
    ]
    res = run_bass_kernel_spmd(nc, in_maps, list(range(NCORES)), trace=_trace)
    out = np.float32(res.results[0]["out"][0, 0])
    if _trace:
        _CACHE["last_result"] = res
    return np.asarray(out, dtype=np.float32)
